# revision 24
# baseline (speedup 1.0000x reference)
"""Trainium2 Bass kernel for nn_AttentionModel (B=4, S=2048, H=8, D=64).

Sharding: 32 (batch, head) pairs split 4-per-core across 8 NeuronCores
(data + head parallel). Each core runs full attention for its 4 heads,
processed as 2 head-pairs so the D=64 contractions can be packed into the
128-row PE array (row tiling) and the 64x64 projections become one
128x128 block-diagonal matmul per pair.

Host-side prep uploads x ALREADY TRANSPOSED per pair (x^T [128=(h1 d|h2
d), S], bf16), so the kernel needs no on-device transposes of x and the
DMA moves 4KB-contiguous rows (full descriptor efficiency).

Per-core pipeline, per head-pair (all emitted chunk-major so the in-order
engines start attention while later DMA chunks are still in flight):
  prep:  q^T/k^T = blockdiag(W^T) @ x^T + b   (bf16 matmul, PSUM -> DVE
         bias-add -> bf16)
         v^T likewise, then PE-transposed back to v' = [v | ones] (bf16,
         ones columns prefilled once; softmax denominator comes out of
         the PV matmul)
  attn:  scores^T[j, i] = k^T_jtile.T @ q^T   (two j-tiles into one
         2-bank PSUM tile)
         ACT Exp (scale=1/8) evacuates PSUM -> bf16 SBUF directly (fused
         exp + copy; ACT is the bottleneck engine at ~90% busy)
         out^T[e|denom, i] += v'_jtile.T @ exp  (accumulated over j)
         PE transpose back to [i, e|denom], DVE reciprocal + scale,
         one batched DMA out per (chunk, head).
  Pair 1's prep is sliced between pair 0's attention blocks, and each
  block's PV/output trails the next block's scores (software pipelining)
  so ACT never idles at block boundaries.

`repeat` runs as a hardware For_i loop so the instruction stream (and
NEFF size / load time) is identical for every repeat count; the device
executes the full body `repeat` times.

Softmax skips the max-subtraction: scores are ~N(0, 0.33), |s| < 10 over
this distribution, exp stays well inside f32 range so the result matches
jax.nn.softmax to f32 precision. x/W/exp staged in bf16 (well inside the
2e-2 rel tolerance; measured ~2e-3).
"""
import numpy as np

B, S, H, D = 4, 2048, 8, 64
NCORES = 8
HPC = 4            # heads per core
NT = S // 128      # 16 s-tiles
NJ = 16            # key tiles of 128
IC = 512           # query-chunk width
NCH = S // IC      # 4 chunks

_cache = {}


def _build(repeat=1):
    import concourse.bacc as bacc
    import concourse.mybir as mybir
    from concourse.tile import TileContext
    from concourse.masks import make_identity
    from concourse.bass import ts

    F32 = mybir.dt.float32
    BF16 = mybir.dt.bfloat16
    AF = mybir.ActivationFunctionType

    nc = bacc.Bacc("TRN2", target_bir_lowering=False, debug=False,
                   num_devices=NCORES)

    # x^T per pair: [pair, (g d), s], bf16, host-pre-transposed
    xq = nc.declare_dram_parameter("xq", [2, 128, S], BF16, isOutput=False)
    xk = nc.declare_dram_parameter("xk", [2, 128, S], BF16, isOutput=False)
    xv = nc.declare_dram_parameter("xv", [2, 128, S], BF16, isOutput=False)
    wq2 = nc.declare_dram_parameter("wq2", [128, 128], BF16, isOutput=False)
    wk2 = nc.declare_dram_parameter("wk2", [128, 128], BF16, isOutput=False)
    wv2 = nc.declare_dram_parameter("wv2", [128, 128], BF16, isOutput=False)
    bq2 = nc.declare_dram_parameter("bq2", [128, 1], F32, isOutput=False)
    bk2 = nc.declare_dram_parameter("bk2", [128, 1], F32, isOutput=False)
    bv2 = nc.declare_dram_parameter("bv2", [128, 1], F32, isOutput=False)
    out_dr = nc.declare_dram_parameter("out", [HPC, S, D], F32, isOutput=True)

    with TileContext(nc) as tc:
        with (
            tc.tile_pool(name="constp", bufs=1) as constp,
            tc.tile_pool(name="xt2p", bufs=2) as xt2p,
            tc.tile_pool(name="qkvp", bufs=2) as qkvp,
            tc.tile_pool(name="vpp", bufs=1) as vpp,
            tc.tile_pool(name="scp", bufs=2) as scp,
            tc.tile_pool(name="obp", bufs=2) as obp,
            tc.tile_pool(name="resp", bufs=3) as resp,
            tc.tile_pool(name="pstp", bufs=2, space="PSUM") as pstp,
            tc.tile_pool(name="psc", bufs=2, space="PSUM") as psc,
            tc.tile_pool(name="psacc", bufs=1, space="PSUM") as psacc,
        ):
            # bf16 identity: transpose cost is keyed on the moving operand
            # (the identity), so bf16 streams at 1 cycle/row.
            identb = constp.tile([128, 128], BF16, name="identb")
            make_identity(nc, identb)
            identf = constp.tile([65, 65], F32, name="identf")
            make_identity(nc, identf)

            w_sb, b_sb = {}, {}
            for nm, wdr, bdr in (("q", wq2, bq2), ("k", wk2, bk2),
                                 ("v", wv2, bv2)):
                w = constp.tile([128, 128], BF16, name=f"w_{nm}")
                nc.sync.dma_start(w[:], wdr[:, :])
                b = constp.tile([128, 1], F32, name=f"b_{nm}")
                nc.sync.dma_start(b[:], bdr[:, :])
                w_sb[nm], b_sb[nm] = w, b

            # persistent v' = [v | ones] tiles, one [128, NJ*65] tile per
            # (pair, head); ones columns filled once via strided memset.
            vprime = [[vpp.tile([128, NJ * 65], BF16, name=f"vp_{p}_{h}",
                                tag=f"vp_{p}_{h}") for h in range(2)]
                      for p in range(2)]
            for p in range(2):
                for h in range(2):
                    nc.gpsimd.memset(
                        vprime[p][h][:].rearrange("s (j e) -> s j e",
                                                  j=NJ)[:, :, 64:65], 1.0)

            qkv = [{} for _ in range(2)]   # per-pair qT2/kT2/vT2 tiles
            xts = [{} for _ in range(2)]   # per-pair x^T tiles

            def prep_chunk(p, m):
                """Chunk-major prep: DMA + projection for s-chunk m of
                every tensor (q/k first), so the in-order engines never
                block early scores on late DMA chunks."""
                for nm, xdr in (("q", xq), ("k", xk), ("v", xv)):
                    if m == 0:
                        xts[p][nm] = xt2p.tile([128, S], BF16,
                                               name=f"xT_{nm}_{p}",
                                               tag=f"xT_{nm}")
                        qkv[p][nm] = qkvp.tile([128, S], BF16,
                                               name=f"{nm}T2_{p}",
                                               tag=f"{nm}T2")
                    nc.sync.dma_start(xts[p][nm][:, ts(m, IC)],
                                      xdr[p, :, ts(m, IC)])
                for nm in "qkv":
                    pp = pstp.tile([128, IC], F32, name=f"pp_{nm}_{p}_{m}",
                                   tag="tp")
                    nc.tensor.matmul(pp[:], w_sb[nm][:],
                                     xts[p][nm][:, ts(m, IC)],
                                     start=True, stop=True)
                    nc.vector.tensor_scalar_add(qkv[p][nm][:, ts(m, IC)],
                                                pp[:], b_sb[nm][:, 0:1])
                # v' tiles for this chunk (vT2 columns 4m..4m+3)
                for jt in range(4 * m, 4 * m + 4):
                    vt = pstp.tile([128, 128], BF16, name=f"vt_{p}_{jt}",
                                   tag="tp")
                    nc.tensor.transpose(vt[:], qkv[p]["v"][:, ts(jt, 128)],
                                        identb[:])
                    for h in range(2):
                        # DVE, not gpsimd: GPSIMD cannot access PSUM
                        nc.vector.tensor_copy(
                            vprime[p][h][:, jt * 65:jt * 65 + 64],
                            vt[:, h * 64:h * 64 + 64])

            sc_of = {}

            def scores_exp(p, c, h, mlist):
                qT2, kT2 = qkv[p]["q"], qkv[p]["k"]
                if (p, c, h) not in sc_of:
                    sc_of[(p, c, h)] = scp.tile([128, NJ * IC], BF16,
                                                name=f"sc_{p}_{c}_{h}",
                                                tag=f"sc{h}")
                sc = sc_of[(p, c, h)]
                for m in mlist:
                    pt2 = psc.tile([128, 2 * IC], F32,
                                   name=f"pt_{p}_{c}_{h}_{m}", tag="pt")
                    for half in range(2):
                        jt = 2 * m + half
                        nc.tensor.matmul(
                            pt2[:, ts(half, IC)],
                            kT2[h * 64:h * 64 + 64, ts(jt, 128)],
                            qT2[h * 64:h * 64 + 64, ts(c, IC)],
                            start=True, stop=True,
                            tile_position=(h * 64, 0),
                        )
                    # fused exp + PSUM->SBUF evacuation on ACT
                    nc.scalar.activation(sc[:, m * 2 * IC:(m + 1) * 2 * IC],
                                         pt2[:], AF.Exp, scale=0.125)

            def pv_out(p, c, h):
                sc = sc_of.pop((p, c, h))
                acc = psacc.tile([65, IC], F32, name=f"acc_{p}_{c}_{h}",
                                 tag=f"acc{h}")
                for jt in range(NJ):
                    nc.tensor.matmul(
                        acc[:], vprime[p][h][:, jt * 65:jt * 65 + 65],
                        sc[:, ts(jt, IC)],
                        start=(jt == 0), stop=(jt == NJ - 1),
                    )
                osb = obp.tile([65, IC], F32, name=f"osb_{p}_{c}_{h}",
                               tag="osb")
                nc.vector.tensor_copy(osb[:], acc[:])
                rsa = resp.tile([128, 4 * 64], F32,
                                name=f"rsa_{p}_{c}_{h}", tag="rsa")
                for u in range(IC // 128):
                    op = pstp.tile([128, 65], F32,
                                   name=f"op_{p}_{c}_{h}_{u}", tag="tp")
                    nc.tensor.transpose(op[:], osb[:, ts(u, 128)],
                                        identf[:])
                    rec = resp.tile([128, 1], F32,
                                    name=f"rec_{p}_{c}_{h}_{u}", tag="rec")
                    nc.vector.reciprocal(rec[:], op[:, 64:65])
                    nc.vector.tensor_scalar_mul(rsa[:, ts(u, 64)],
                                                op[:, 0:64],
                                                rec[:, 0:1])
                g = 2 * p + h
                nc.sync.dma_start(
                    out_dr[g, c * IC:(c + 1) * IC, :]
                    .rearrange("(u s) d -> s u d", u=4),
                    rsa[:].rearrange("s (u d) -> s u d", u=4),
                )

            # `repeat` as a hardware loop: same NEFF for any repeat count.
            with tc.For_i(0, repeat, 1):
                for m in range(NCH):
                    prep_chunk(0, m)
                    scores_exp(0, 0, 0, [2 * m, 2 * m + 1])
                    scores_exp(0, 0, 1, [2 * m, 2 * m + 1])
                pv_out(0, 0, 0)

                p1_slices = {
                    (0, 1, 0): lambda: prep_chunk(1, 0),
                    (0, 2, 0): lambda: prep_chunk(1, 1),
                    (0, 3, 0): lambda: (prep_chunk(1, 2),
                                        prep_chunk(1, 3)),
                }
                seq = ([(0, c, h) for c in range(1, NCH) for h in range(2)]
                       + [(1, c, h) for c in range(NCH) for h in range(2)])
                prev = (0, 0, 1)
                for blk in seq:
                    if blk in p1_slices:
                        p1_slices[blk]()
                    scores_exp(*blk, range(NJ // 2))
                    pv_out(*prev)
                    prev = blk
                pv_out(*prev)

    nc.compile()
    return nc


def _prep_inputs(query, key, value, Wq, bq, Wk, bk, Wv, bv):
    """Host-side sharding/layout prep. Returns per-core input maps."""
    import ml_dtypes

    def head_major_T(x):
        # [B,S,H,D] -> head-major x^T: [B*H/2 pairs, 2, 128=(g d), S]
        xt = (np.asarray(x, np.float32).transpose(0, 2, 3, 1)
              .reshape(B * H // 2, 2 * D, S))           # [(b h2), (g d), s]
        return np.ascontiguousarray(xt).astype(ml_dtypes.bfloat16)

    qh, kh, vh = head_major_T(query), head_major_T(key), head_major_T(value)

    def blockdiag(W):
        Wt = np.asarray(W, np.float32).T  # [d, e]
        W2 = np.zeros((128, 128), np.float32)
        W2[:64, :64] = Wt
        W2[64:, 64:] = Wt
        return W2.astype(ml_dtypes.bfloat16)

    def bias2(b):
        return np.concatenate([np.asarray(b, np.float32)] * 2).reshape(128, 1)

    shared = dict(wq2=blockdiag(Wq), wk2=blockdiag(Wk), wv2=blockdiag(Wv),
                  bq2=bias2(bq), bk2=bias2(bk), bv2=bias2(bv))
    in_maps = []
    for c in range(NCORES):
        sl = slice(c * 2, (c + 1) * 2)   # 2 head-pairs per core
        in_maps.append(dict(xq=np.ascontiguousarray(qh[sl]),
                            xk=np.ascontiguousarray(kh[sl]),
                            xv=np.ascontiguousarray(vh[sl]), **shared))
    return in_maps


def kernel(query, key, value, Wq, bq, Wk, bk, Wv, bv):
    from concourse.bass_utils import run_bass_kernel_spmd

    if "nc" not in _cache:
        _cache["nc"] = _build()
    nc = _cache["nc"]

    in_maps = _prep_inputs(query, key, value, Wq, bq, Wk, bk, Wv, bv)
    res = run_bass_kernel_spmd(nc, in_maps, list(range(NCORES)))
    out = np.stack([res.results[c]["out"] for c in range(NCORES)])  # [8,4,S,D]
    out = out.reshape(B * H, S, D).reshape(B, H, S, D).transpose(0, 2, 1, 3)
    return np.ascontiguousarray(out)


# revision 27
# speedup vs baseline: 5.5342x; 5.5342x over previous
"""Trainium2 Bass kernel for nn_AttentionModel (B=4, S=2048, H=8, D=64).

Sharding: 32 (batch, head) pairs split 4-per-core across 8 NeuronCores
(data + head parallel). Each core runs full attention for its 4 heads,
processed as 2 head-pairs so the D=64 contractions can be packed into the
128-row PE array (row tiling) and the 64x64 projections become one
128x128 block-diagonal matmul per pair.

Host-side prep uploads x ALREADY TRANSPOSED per pair (x^T [128=(h1 d|h2
d), S], bf16), so the kernel needs no on-device transposes of x and the
DMA moves 4KB-contiguous rows (full descriptor efficiency).

Per-core pipeline, per head-pair (all emitted chunk-major so the in-order
engines start attention while later DMA chunks are still in flight):
  prep:  q^T/k^T = blockdiag(W^T) @ x^T + b   (bf16 matmul, PSUM -> DVE
         bias-add -> bf16)
         v^T likewise, then PE-transposed back to v' = [v | ones] (bf16,
         ones columns prefilled once; softmax denominator comes out of
         the PV matmul)
  attn:  scores^T[j, i] = k^T_jtile.T @ q^T   (two j-tiles into one
         2-bank PSUM tile)
         ACT Exp (scale=1/8) evacuates PSUM -> bf16 SBUF directly (fused
         exp + copy; ACT is the bottleneck engine at ~90% busy)
         out^T[e|denom, i] += v'_jtile.T @ exp  (accumulated over j)
         PE transpose back to [i, e|denom], DVE reciprocal + scale,
         one batched DMA out per (chunk, head).
  Pair 1's prep is sliced between pair 0's attention blocks, and each
  block's PV/output trails the next block's scores (software pipelining)
  so ACT never idles at block boundaries.

`repeat` runs as a hardware For_i loop so the instruction stream (and
NEFF size / load time) is identical for every repeat count; the device
executes the full body `repeat` times.

Softmax skips the max-subtraction: scores are ~N(0, 0.33), |s| < 10 over
this distribution, exp stays well inside f32 range so the result matches
jax.nn.softmax to f32 precision. x/W/exp staged in bf16 (well inside the
2e-2 rel tolerance; measured ~2e-3).
"""
import numpy as np

B, S, H, D = 4, 2048, 8, 64
NCORES = 8
HPC = 4            # heads per core
NT = S // 128      # 16 s-tiles
NJ = 16            # key tiles of 128
IC = 512           # query-chunk width
NCH = S // IC      # 4 chunks

_cache = {}


def _build(repeat=1):
    import concourse.bacc as bacc
    import concourse.mybir as mybir
    from concourse.tile import TileContext
    from concourse.masks import make_identity
    from concourse.bass import ts

    F32 = mybir.dt.float32
    BF16 = mybir.dt.bfloat16
    AF = mybir.ActivationFunctionType

    nc = bacc.Bacc("TRN2", target_bir_lowering=False, debug=False,
                   num_devices=NCORES)

    # x^T per pair: [pair, (g d), s], bf16, host-pre-transposed
    xq = nc.declare_dram_parameter("xq", [2, 128, S], BF16, isOutput=False)
    xk = nc.declare_dram_parameter("xk", [2, 128, S], BF16, isOutput=False)
    xv = nc.declare_dram_parameter("xv", [2, 128, S], BF16, isOutput=False)
    wq2 = nc.declare_dram_parameter("wq2", [128, 128], BF16, isOutput=False)
    wk2 = nc.declare_dram_parameter("wk2", [128, 128], BF16, isOutput=False)
    wv2 = nc.declare_dram_parameter("wv2", [128, 128], BF16, isOutput=False)
    bq2 = nc.declare_dram_parameter("bq2", [128, 1], F32, isOutput=False)
    bk2 = nc.declare_dram_parameter("bk2", [128, 1], F32, isOutput=False)
    bv2 = nc.declare_dram_parameter("bv2", [128, 1], F32, isOutput=False)
    out_dr = nc.declare_dram_parameter("out", [HPC, S, D], BF16, isOutput=True)

    with TileContext(nc) as tc:
        with (
            tc.tile_pool(name="constp", bufs=1) as constp,
            tc.tile_pool(name="xt2p", bufs=2) as xt2p,
            tc.tile_pool(name="qkvp", bufs=2) as qkvp,
            tc.tile_pool(name="vpp", bufs=1) as vpp,
            tc.tile_pool(name="scp", bufs=2) as scp,
            tc.tile_pool(name="obp", bufs=2) as obp,
            tc.tile_pool(name="resp", bufs=3) as resp,
            tc.tile_pool(name="pstp", bufs=2, space="PSUM") as pstp,
            tc.tile_pool(name="psc", bufs=2, space="PSUM") as psc,
            tc.tile_pool(name="psacc", bufs=1, space="PSUM") as psacc,
        ):
            # bf16 identity: transpose cost is keyed on the moving operand
            # (the identity), so bf16 streams at 1 cycle/row.
            identb = constp.tile([128, 128], BF16, name="identb")
            make_identity(nc, identb)
            identf = constp.tile([65, 65], F32, name="identf")
            make_identity(nc, identf)

            w_sb, b_sb = {}, {}
            for nm, wdr, bdr in (("q", wq2, bq2), ("k", wk2, bk2),
                                 ("v", wv2, bv2)):
                w = constp.tile([128, 128], BF16, name=f"w_{nm}")
                nc.sync.dma_start(w[:], wdr[:, :])
                b = constp.tile([128, 1], F32, name=f"b_{nm}")
                nc.sync.dma_start(b[:], bdr[:, :])
                w_sb[nm], b_sb[nm] = w, b

            # persistent v' = [v | ones] tiles, one [128, NJ*65] tile per
            # (pair, head); ones columns filled once via strided memset.
            vprime = [[vpp.tile([128, NJ * 65], BF16, name=f"vp_{p}_{h}",
                                tag=f"vp_{p}_{h}") for h in range(2)]
                      for p in range(2)]
            for p in range(2):
                for h in range(2):
                    nc.gpsimd.memset(
                        vprime[p][h][:].rearrange("s (j e) -> s j e",
                                                  j=NJ)[:, :, 64:65], 1.0)

            qkv = [{} for _ in range(2)]   # per-pair qT2/kT2/vT2 tiles
            xts = [{} for _ in range(2)]   # per-pair x^T tiles

            def prep_chunk(p, m):
                """Chunk-major prep: DMA + projection for s-chunk m of
                every tensor (q/k first), so the in-order engines never
                block early scores on late DMA chunks."""
                for nm, xdr in (("q", xq), ("k", xk), ("v", xv)):
                    if m == 0:
                        xts[p][nm] = xt2p.tile([128, S], BF16,
                                               name=f"xT_{nm}_{p}",
                                               tag=f"xT_{nm}")
                        qkv[p][nm] = qkvp.tile([128, S], BF16,
                                               name=f"{nm}T2_{p}",
                                               tag=f"{nm}T2")
                    nc.sync.dma_start(xts[p][nm][:, ts(m, IC)],
                                      xdr[p, :, ts(m, IC)])
                for nm in "qkv":
                    pp = pstp.tile([128, IC], F32, name=f"pp_{nm}_{p}_{m}",
                                   tag="tp")
                    nc.tensor.matmul(pp[:], w_sb[nm][:],
                                     xts[p][nm][:, ts(m, IC)],
                                     start=True, stop=True)
                    nc.vector.tensor_scalar_add(qkv[p][nm][:, ts(m, IC)],
                                                pp[:], b_sb[nm][:, 0:1])
                # v' tiles for this chunk (vT2 columns 4m..4m+3)
                for jt in range(4 * m, 4 * m + 4):
                    vt = pstp.tile([128, 128], BF16, name=f"vt_{p}_{jt}",
                                   tag="tp")
                    nc.tensor.transpose(vt[:], qkv[p]["v"][:, ts(jt, 128)],
                                        identb[:])
                    for h in range(2):
                        # DVE, not gpsimd: GPSIMD cannot access PSUM
                        nc.vector.tensor_copy(
                            vprime[p][h][:, jt * 65:jt * 65 + 64],
                            vt[:, h * 64:h * 64 + 64])

            sc_of = {}

            def scores_exp(p, c, h, mlist):
                qT2, kT2 = qkv[p]["q"], qkv[p]["k"]
                if (p, c, h) not in sc_of:
                    sc_of[(p, c, h)] = scp.tile([128, NJ * IC], BF16,
                                                name=f"sc_{p}_{c}_{h}",
                                                tag=f"sc{h}")
                sc = sc_of[(p, c, h)]
                for m in mlist:
                    pt2 = psc.tile([128, 2 * IC], F32,
                                   name=f"pt_{p}_{c}_{h}_{m}", tag="pt")
                    for half in range(2):
                        jt = 2 * m + half
                        nc.tensor.matmul(
                            pt2[:, ts(half, IC)],
                            kT2[h * 64:h * 64 + 64, ts(jt, 128)],
                            qT2[h * 64:h * 64 + 64, ts(c, IC)],
                            start=True, stop=True,
                            tile_position=(h * 64, 0),
                        )
                    # fused exp + PSUM->SBUF evacuation on ACT
                    nc.scalar.activation(sc[:, m * 2 * IC:(m + 1) * 2 * IC],
                                         pt2[:], AF.Exp, scale=0.125)

            def pv_out(p, c, h):
                sc = sc_of.pop((p, c, h))
                acc = psacc.tile([65, IC], F32, name=f"acc_{p}_{c}_{h}",
                                 tag=f"acc{h}")
                for jt in range(NJ):
                    nc.tensor.matmul(
                        acc[:], vprime[p][h][:, jt * 65:jt * 65 + 65],
                        sc[:, ts(jt, IC)],
                        start=(jt == 0), stop=(jt == NJ - 1),
                    )
                osb = obp.tile([65, IC], F32, name=f"osb_{p}_{c}_{h}",
                               tag="osb")
                nc.vector.tensor_copy(osb[:], acc[:])
                rsa = resp.tile([128, 4 * 64], BF16,
                                name=f"rsa_{p}_{c}_{h}", tag="rsa")
                for u in range(IC // 128):
                    op = pstp.tile([128, 65], F32,
                                   name=f"op_{p}_{c}_{h}_{u}", tag="tp")
                    nc.tensor.transpose(op[:], osb[:, ts(u, 128)],
                                        identf[:])
                    rec = resp.tile([128, 1], F32,
                                    name=f"rec_{p}_{c}_{h}_{u}", tag="rec")
                    nc.vector.reciprocal(rec[:], op[:, 64:65])
                    nc.vector.tensor_scalar_mul(rsa[:, ts(u, 64)],
                                                op[:, 0:64],
                                                rec[:, 0:1])
                g = 2 * p + h
                nc.sync.dma_start(
                    out_dr[g, c * IC:(c + 1) * IC, :]
                    .rearrange("(u s) d -> s u d", u=4),
                    rsa[:].rearrange("s (u d) -> s u d", u=4),
                )

            # `repeat` as a hardware loop: same NEFF for any repeat count.
            with tc.For_i(0, repeat, 1):
                for m in range(NCH):
                    prep_chunk(0, m)
                    scores_exp(0, 0, 0, [2 * m, 2 * m + 1])
                    scores_exp(0, 0, 1, [2 * m, 2 * m + 1])
                pv_out(0, 0, 0)

                p1_slices = {
                    (0, 1, 0): lambda: prep_chunk(1, 0),
                    (0, 2, 0): lambda: prep_chunk(1, 1),
                    (0, 3, 0): lambda: (prep_chunk(1, 2),
                                        prep_chunk(1, 3)),
                }
                seq = ([(0, c, h) for c in range(1, NCH) for h in range(2)]
                       + [(1, c, h) for c in range(NCH) for h in range(2)])
                prev = (0, 0, 1)
                for blk in seq:
                    if blk in p1_slices:
                        p1_slices[blk]()
                    scores_exp(*blk, range(NJ // 2))
                    pv_out(*prev)
                    prev = blk
                pv_out(*prev)

    nc.compile()
    return nc


def _prep_inputs(query, key, value, Wq, bq, Wk, bk, Wv, bv):
    """Host-side sharding/layout prep. Returns per-core input maps."""
    import ml_dtypes

    def head_major_T(x):
        # [B,S,H,D] -> head-major x^T: [B*H/2 pairs, 2, 128=(g d), S]
        xt = (np.asarray(x, np.float32).transpose(0, 2, 3, 1)
              .reshape(B * H // 2, 2 * D, S))           # [(b h2), (g d), s]
        return np.ascontiguousarray(xt).astype(ml_dtypes.bfloat16)

    qh, kh, vh = head_major_T(query), head_major_T(key), head_major_T(value)

    def blockdiag(W):
        Wt = np.asarray(W, np.float32).T  # [d, e]
        W2 = np.zeros((128, 128), np.float32)
        W2[:64, :64] = Wt
        W2[64:, 64:] = Wt
        return W2.astype(ml_dtypes.bfloat16)

    def bias2(b):
        return np.concatenate([np.asarray(b, np.float32)] * 2).reshape(128, 1)

    shared = dict(wq2=blockdiag(Wq), wk2=blockdiag(Wk), wv2=blockdiag(Wv),
                  bq2=bias2(bq), bk2=bias2(bk), bv2=bias2(bv))
    in_maps = []
    for c in range(NCORES):
        sl = slice(c * 2, (c + 1) * 2)   # 2 head-pairs per core
        in_maps.append(dict(xq=np.ascontiguousarray(qh[sl]),
                            xk=np.ascontiguousarray(kh[sl]),
                            xv=np.ascontiguousarray(vh[sl]), **shared))
    return in_maps


def kernel(query, key, value, Wq, bq, Wk, bk, Wv, bv):
    from concourse.bass_utils import run_bass_kernel_spmd

    if "nc" not in _cache:
        _cache["nc"] = _build()
    nc = _cache["nc"]

    in_maps = _prep_inputs(query, key, value, Wq, bq, Wk, bk, Wv, bv)
    res = run_bass_kernel_spmd(nc, in_maps, list(range(NCORES)))
    out = np.stack([np.asarray(res.results[c]["out"], np.float32)
                    for c in range(NCORES)])              # [8,4,S,D]
    out = out.reshape(B * H, S, D).reshape(B, H, S, D).transpose(0, 2, 1, 3)
    return np.ascontiguousarray(out)
